# revision 18
# baseline (speedup 1.0000x reference)
"""Multi-head self-attention Trainium2 kernel (8-core SPMD, full IO).

Problem: x:(2,2048,1024) f32; Wq/Wk/Wv/Wo:(1024,1024); bo:(1024,)
  out = softmax((xWq)(xWk)^T / 8) (xWv) reshaped @ Wo + bo

Sharding: data parallel on batch N=2 x tensor parallel on 16 heads in
4 groups of 4 heads.  Core c handles batch c//4, heads [4*(c%4), 4*(c%4)+4).
Each core computes a partial fc_out product (2048,1024); the host sums the
4 head-group partials per batch and adds the bias.

On-chip layout (per core):
  xT   (1024,2048)  x[n]^T, embed on partitions (8 chunks of 128)
  Q^T/K^T stored as [128, 2, 2048] (dims-chunk on partitions, tokens free)
  V    stored as [128(tokens), 16, 4, 65]; col 64 = ones (denominator trick)
  scores are computed TRANSPOSED: S^T[k,q] so that exp runs on ACT and the
  softmax denominator falls out of the ones-column of V during the O^T
  accumulation (row 64 of the [65,512] psum).  No max subtraction: scores
  are ~N(0,1), bounded well inside fp32 exp range (as in the reference,
  which subtracts max only for stability, not value).
"""

import os

import numpy as np

import concourse.bass as bass
import concourse.tile as tile
from concourse import bacc, mybir
from concourse import bass_utils

F32 = mybir.dt.float32

EMBED = 1024
SEQ = 2048
NB = 2  # batch
HEADS = 16
D = 64  # head dim
NCORES = 8
GROUPS = 4  # head groups (tensor parallel)
HG = HEADS // GROUPS  # heads per core = 4
DG = HG * D  # dims per core = 256

# matmul operand dtype: float32 (exact, 1/4 rate) or float32r (full rate,
# reduced-precision multiplies).  Overridable for experiments.
_MM_DTYPE_NAME = os.environ.get("MHA_MM_DTYPE", "float32r")
MM_DTYPE = getattr(mybir.dt, _MM_DTYPE_NAME)

# set by run_cores(); test.py reads exec_time_ns from here
LAST_RESULTS = None
_CACHED_NC = {}


def _c(ap):
    """Bitcast an fp32 AP to the matmul operand dtype."""
    if MM_DTYPE == F32:
        return ap
    return ap.bitcast(MM_DTYPE)


def build_nc():
    nc = bacc.Bacc("TRN2", target_bir_lowering=False, debug=False,
                   num_devices=NCORES)

    xT = nc.dram_tensor("xT", (EMBED, SEQ), F32, kind="ExternalInput").ap()
    wq = nc.dram_tensor("wq", (EMBED, DG), F32, kind="ExternalInput").ap()
    wk = nc.dram_tensor("wk", (EMBED, DG), F32, kind="ExternalInput").ap()
    wv = nc.dram_tensor("wv", (EMBED, DG), F32, kind="ExternalInput").ap()
    wo = nc.dram_tensor("wo", (DG, EMBED), F32, kind="ExternalInput").ap()
    y = nc.dram_tensor("y", (SEQ, EMBED), F32, kind="ExternalOutput").ap()
    # DRAM bounce buffers for the softmax denominators: SBUF sources can't be
    # partition-broadcast by DMA, DRAM sources can.
    den_dram = nc.dram_tensor("den_scratch", (HG, SEQ), F32).ap()
    rden_dram = nc.dram_tensor("rden_scratch", (HG, SEQ), F32).ap()

    KC = EMBED // 128  # 8 contraction chunks for projections

    with tile.TileContext(nc) as tc:
        with (
            tc.tile_pool(name="weights", bufs=1) as wpool,
            tc.tile_pool(name="qk", bufs=1) as qkpool,
            tc.tile_pool(name="vpool", bufs=1) as vpool,
            tc.tile_pool(name="otpool", bufs=1) as otpool,
            tc.tile_pool(name="xchunk", bufs=2) as xpool,
            tc.tile_pool(name="epool", bufs=4) as epool,
            tc.tile_pool(name="stage", bufs=2) as stpool,
            tc.tile_pool(name="den", bufs=1) as denpool,
            tc.tile_pool(name="rbc", bufs=2) as rbcpool,
            tc.tile_pool(name="ystage", bufs=3) as ypool,
            tc.tile_pool(name="psum", bufs=2, space="PSUM") as pspool,
            tc.tile_pool(name="psum_o", bufs=2, space="PSUM") as popool,
        ):
            # ---- load weights ----
            wq_sb = wpool.tile([128, KC, DG], F32)
            wk_sb = wpool.tile([128, KC, DG], F32)
            wv_sb = wpool.tile([128, KC, DG], F32)
            wo_sb = wpool.tile([128, DG // 128, EMBED], F32)
            nc.sync.dma_start(out=wq_sb, in_=wq.rearrange("(c p) n -> p c n", p=128))
            nc.sync.dma_start(out=wk_sb, in_=wk.rearrange("(c p) n -> p c n", p=128))
            nc.sync.dma_start(out=wv_sb, in_=wv.rearrange("(c p) n -> p c n", p=128))
            nc.sync.dma_start(out=wo_sb, in_=wo.rearrange("(c p) n -> p c n", p=128))

            QT = qkpool.tile([128, 2, SEQ], F32)  # [dim-chunk part, mt, tokens]
            KT = qkpool.tile([128, 2, SEQ], F32)
            V = vpool.tile([128, 16, HG, D + 1], F32)  # [tok part, tok-tile, head, d+1]
            nc.vector.memset(V[:, :, :, D:D + 1], 1.0)

            xTr = xT.rearrange("(c p) s -> p c s", p=128)

            # ---- phase 1: projections, one 256-token chunk at a time ----
            TCH = 256
            for tcb in range(SEQ // TCH):
                xc = xpool.tile([128, KC, TCH], F32)
                nc.sync.dma_start(out=xc, in_=xTr[:, :, tcb * TCH:(tcb + 1) * TCH])

                # Q^T and K^T chunks: [dims 128, tokens TCH]
                for wsb, dst in ((wq_sb, QT), (wk_sb, KT)):
                    for mt in range(2):
                        ps = pspool.tile([128, 512], F32)
                        for kc in range(KC):
                            nc.tensor.matmul(
                                ps[:, 0:TCH],
                                _c(wsb[:, kc, mt * 128:(mt + 1) * 128]),
                                _c(xc[:, kc, :]),
                                start=(kc == 0),
                                stop=(kc == KC - 1),
                            )
                        nc.vector.tensor_copy(
                            out=dst[:, mt, tcb * TCH:(tcb + 1) * TCH],
                            in_=ps[:, 0:TCH])

                # V chunks: [tokens 128, dims 256]
                for ti in range(TCH // 128):
                    tt = tcb * (TCH // 128) + ti
                    ps = pspool.tile([128, 512], F32)
                    for kc in range(KC):
                        nc.tensor.matmul(
                            ps[:, 0:DG],
                            _c(xc[:, kc, ti * 128:(ti + 1) * 128]),
                            _c(wv_sb[:, kc, :]),
                            start=(kc == 0),
                            stop=(kc == KC - 1),
                        )
                    for h in range(HG):
                        nc.vector.tensor_copy(
                            out=V[:, tt, h, 0:D], in_=ps[:, h * D:(h + 1) * D])

            # ---- phase 2: attention (scores transposed, head pairs) ----
            # OT2[p, hm, q]: partition p = 64*j + d for head h = 2*hm + j.
            # This matches wo_sb's row layout so fc_out contracts K=128/pair.
            OT2 = otpool.tile([128, 2, SEQ], F32)
            QC = 1024  # q-chunk: one [128, QC] psum = 2 banks, one exp inst

            for hm in range(2):
                for qc in range(SEQ // QC):
                    qs = slice(qc * QC, (qc + 1) * QC)
                    po = [popool.tile([D + 1, QC], F32, name="po", tag="po")
                          for _ in range(2)]
                    for m in range(SEQ // 128):
                        es = []
                        for j in range(2):  # paired heads -> concurrent MMs
                            ps = pspool.tile([128, QC], F32)
                            for ha in range(QC // 512):
                                nc.tensor.matmul(
                                    ps[:, ha * 512:(ha + 1) * 512],
                                    _c(KT[j * D:(j + 1) * D, hm,
                                          m * 128:(m + 1) * 128]),
                                    _c(QT[j * D:(j + 1) * D, hm,
                                          qc * QC + ha * 512:
                                          qc * QC + (ha + 1) * 512]),
                                    start=True,
                                    stop=True,
                                )
                            e = epool.tile([128, QC], F32)
                            nc.scalar.activation(
                                out=e, in_=ps,
                                func=mybir.ActivationFunctionType.Exp,
                                scale=1.0 / np.sqrt(D),
                            )
                            es.append(e)
                        for j in range(2):
                            for ha in range(QC // 512):
                                nc.tensor.matmul(
                                    po[j][:, ha * 512:(ha + 1) * 512],
                                    _c(V[:, m, 2 * hm + j, :]),
                                    _c(es[j][:, ha * 512:(ha + 1) * 512]),
                                    start=(m == 0),
                                    stop=(m == SEQ // 128 - 1),
                                )
                    for j in range(2):
                        h = 2 * hm + j
                        st = stpool.tile([D + 1, QC], F32)
                        nc.vector.tensor_copy(out=st, in_=po[j])
                        nc.sync.dma_start(
                            out=OT2[j * D:(j + 1) * D, hm, qs], in_=st[0:D, :])
                        nc.sync.dma_start(
                            out=den_dram[h:h + 1, qs], in_=st[D:D + 1, :])

            # reciprocal, reshaped to use all 128 partitions (free dim 64)
            rsm = denpool.tile([128, HG * SEQ // 128], F32)
            den_r = den_dram.rearrange("h (a b) -> (h a) b", a=32)
            rden_r = rden_dram.rearrange("h (a b) -> (h a) b", a=32)
            nc.sync.dma_start(out=rsm, in_=den_r)
            nc.vector.reciprocal(out=rsm, in_=rsm)
            nc.sync.dma_start(out=rden_r, in_=rsm)

            # normalize O^T rows by 1/denominator (broadcast across partitions)
            for hm in range(2):
                rb = rbcpool.tile([128, SEQ], F32)
                for j in range(2):
                    nc.sync.dma_start(
                        out=rb[j * D:(j + 1) * D, :],
                        in_=rden_dram[2 * hm + j:2 * hm + j + 1, :]
                        .to_broadcast((D, SEQ)))
                nc.vector.tensor_mul(OT2[:, hm, :], OT2[:, hm, :], rb)

            # ---- phase 3: partial fc_out  y = sum_h O_h @ Wo_h (K=128/pair) ----
            for tt in range(SEQ // 128):
                for nch in range(EMBED // 512):
                    ps = pspool.tile([128, QC], F32)
                    for hm in range(2):
                        nc.tensor.matmul(
                            ps[:, 0:512],
                            _c(OT2[:, hm, tt * 128:(tt + 1) * 128]),
                            _c(wo_sb[:, hm, nch * 512:(nch + 1) * 512]),
                            start=(hm == 0),
                            stop=(hm == 1),
                        )
                    ys = ypool.tile([128, 512], F32)
                    nc.vector.tensor_copy(out=ys, in_=ps[:, 0:512])
                    nc.sync.dma_start(
                        out=y[tt * 128:(tt + 1) * 128, nch * 512:(nch + 1) * 512],
                        in_=ys)

    nc.compile()
    return nc


def shard_inputs(x, Wv, Wk, Wq, Wo):
    """Build the 8 per-core input maps."""
    in_maps = []
    for c in range(NCORES):
        n, g = divmod(c, GROUPS)
        cols = slice(g * DG, (g + 1) * DG)
        in_maps.append({
            "xT": np.ascontiguousarray(np.asarray(x[n], dtype=np.float32).T),
            "wq": np.ascontiguousarray(np.asarray(Wq, np.float32)[:, cols]),
            "wk": np.ascontiguousarray(np.asarray(Wk, np.float32)[:, cols]),
            "wv": np.ascontiguousarray(np.asarray(Wv, np.float32)[:, cols]),
            "wo": np.ascontiguousarray(np.asarray(Wo, np.float32)[cols, :]),
        })
    return in_maps


def kernel(x, Wv, Wk, Wq, Wo, bo):
    global LAST_RESULTS
    x = np.asarray(x, np.float32)
    in_maps = shard_inputs(x, Wv, Wk, Wq, Wo)

    if "nc" not in _CACHED_NC:
        _CACHED_NC["nc"] = build_nc()
    nc = _CACHED_NC["nc"]

    trace = os.environ.get("MHA_TRACE", "0") == "1"
    res = bass_utils.run_bass_kernel_spmd(
        nc, in_maps, core_ids=list(range(NCORES)), trace=trace)
    LAST_RESULTS = res

    bo = np.asarray(bo, np.float32)
    out = np.empty((NB, SEQ, EMBED), np.float32)
    for n in range(NB):
        acc = res.results[n * GROUPS]["y"].astype(np.float32).copy()
        for g in range(1, GROUPS):
            acc += res.results[n * GROUPS + g]["y"]
        out[n] = acc + bo[None, :]
    return out


# revision 21
# speedup vs baseline: 2.1490x; 2.1490x over previous
"""Multi-head self-attention Trainium2 kernel (8-core SPMD, full IO).

Problem: x:(2,2048,1024) f32; Wq/Wk/Wv/Wo:(1024,1024); bo:(1024,)
  out = softmax((xWq)(xWk)^T / 8) (xWv) reshaped @ Wo + bo

Sharding: data parallel on batch N=2 x tensor parallel on 16 heads in
4 groups of 4 heads.  Core c handles batch c//4, heads [4*(c%4), 4*(c%4)+4).
Each core computes a partial fc_out product (2048,1024); the host sums the
4 head-group partials per batch and adds the bias.

On-chip layout (per core):
  xT   (1024,2048)  x[n]^T, embed on partitions (8 chunks of 128)
  Q^T/K^T stored as [128, 2, 2048] (dims-chunk on partitions, tokens free)
  V    stored as [128(tokens), 16, 4, 65]; col 64 = ones (denominator trick)
  scores are computed TRANSPOSED: S^T[k,q] so that exp runs on ACT and the
  softmax denominator falls out of the ones-column of V during the O^T
  accumulation (row 64 of the [65,512] psum).  No max subtraction: scores
  are ~N(0,1), bounded well inside fp32 exp range (as in the reference,
  which subtracts max only for stability, not value).
"""

import os

import numpy as np

import concourse.bass as bass
import concourse.tile as tile
from concourse import bacc, mybir
from concourse import bass_utils

F32 = mybir.dt.float32

EMBED = 1024
SEQ = 2048
NB = 2  # batch
HEADS = 16
D = 64  # head dim
NCORES = 8
GROUPS = 4  # head groups (tensor parallel)
HG = HEADS // GROUPS  # heads per core = 4
DG = HG * D  # dims per core = 256

# matmul operand dtype: float32 (exact, 1/4 rate) or float32r (full rate,
# reduced-precision multiplies).  Overridable for experiments.
_MM_DTYPE_NAME = os.environ.get("MHA_MM_DTYPE", "float32r")
MM_DTYPE = getattr(mybir.dt, _MM_DTYPE_NAME)

# set by run_cores(); test.py reads exec_time_ns from here
LAST_RESULTS = None
_CACHED_NC = {}


MD = MM_DTYPE  # dtype of matmul-feeding tiles (storage-compatible with f32)


def build_nc():
    nc = bacc.Bacc("TRN2", target_bir_lowering=False, debug=False,
                   num_devices=NCORES)

    xT = nc.dram_tensor("xT", (EMBED, SEQ), F32, kind="ExternalInput").ap()
    wq = nc.dram_tensor("wq", (EMBED, DG), F32, kind="ExternalInput").ap()
    wk = nc.dram_tensor("wk", (EMBED, DG), F32, kind="ExternalInput").ap()
    wv = nc.dram_tensor("wv", (EMBED, DG), F32, kind="ExternalInput").ap()
    wo = nc.dram_tensor("wo", (DG, EMBED), F32, kind="ExternalInput").ap()
    y = nc.dram_tensor("y", (SEQ, EMBED), F32, kind="ExternalOutput").ap()
    # DRAM bounce buffers for the softmax denominators: SBUF sources can't be
    # partition-broadcast by DMA, DRAM sources can.
    den_dram = nc.dram_tensor("den_scratch", (HG, SEQ), F32).ap()
    rden_dram = nc.dram_tensor("rden_scratch", (HG, SEQ), F32).ap()

    KC = EMBED // 128  # 8 contraction chunks for projections

    with tile.TileContext(nc) as tc:
        with (
            tc.tile_pool(name="weights", bufs=1) as wpool,
            tc.tile_pool(name="qk", bufs=1) as qkpool,
            tc.tile_pool(name="vpool", bufs=1) as vpool,
            tc.tile_pool(name="otpool", bufs=1) as otpool,
            tc.tile_pool(name="xchunk", bufs=2) as xpool,
            tc.tile_pool(name="epool", bufs=4) as epool,
            tc.tile_pool(name="stage", bufs=2) as stpool,
            tc.tile_pool(name="den", bufs=1) as denpool,
            tc.tile_pool(name="rbc", bufs=2) as rbcpool,
            tc.tile_pool(name="ystage", bufs=3) as ypool,
            tc.tile_pool(name="psum", bufs=2, space="PSUM") as pspool,
            tc.tile_pool(name="psum_o", bufs=2, space="PSUM") as popool,
        ):
            # ---- load weights ----
            wq_sb = wpool.tile([128, KC, DG], MD)
            wk_sb = wpool.tile([128, KC, DG], MD)
            wv_sb = wpool.tile([128, KC, DG], MD)
            wo_sb = wpool.tile([128, DG // 128, EMBED], MD)
            nc.sync.dma_start(out=wq_sb, in_=wq.bitcast(MD).rearrange("(c p) n -> p c n", p=128))
            nc.sync.dma_start(out=wk_sb, in_=wk.bitcast(MD).rearrange("(c p) n -> p c n", p=128))
            nc.sync.dma_start(out=wv_sb, in_=wv.bitcast(MD).rearrange("(c p) n -> p c n", p=128))
            nc.sync.dma_start(out=wo_sb, in_=wo.bitcast(MD).rearrange("(c p) n -> p c n", p=128))

            QT = qkpool.tile([128, 2, SEQ], MD)  # [dim-chunk part, mt, tokens]
            KT = qkpool.tile([128, 2, SEQ], MD)
            V = vpool.tile([128, 16, HG, D + 1], MD)  # [tok part, tok-tile, head, d+1]
            nc.vector.memset(V[:, :, :, D:D + 1].bitcast(F32), 1.0)

            xTr = xT.bitcast(MD).rearrange("(c p) s -> p c s", p=128)

            # ---- phase 1: projections, one 256-token chunk at a time ----
            TCH = 256
            for tcb in range(SEQ // TCH):
                xc = xpool.tile([128, KC, TCH], MD)
                nc.sync.dma_start(out=xc, in_=xTr[:, :, tcb * TCH:(tcb + 1) * TCH])

                # Q^T and K^T chunks: [dims 128, tokens TCH]
                for wsb, dst in ((wq_sb, QT), (wk_sb, KT)):
                    for mt in range(2):
                        ps = pspool.tile([128, 512], F32)
                        for kc in range(KC):
                            nc.tensor.matmul(
                                ps[:, 0:TCH],
                                wsb[:, kc, mt * 128:(mt + 1) * 128],
                                xc[:, kc, :],
                                start=(kc == 0),
                                stop=(kc == KC - 1),
                            )
                        nc.vector.tensor_copy(
                            out=dst[:, mt, tcb * TCH:(tcb + 1) * TCH],
                            in_=ps[:, 0:TCH])

                # V chunks: [tokens 128, dims 256]
                for ti in range(TCH // 128):
                    tt = tcb * (TCH // 128) + ti
                    ps = pspool.tile([128, 512], F32)
                    for kc in range(KC):
                        nc.tensor.matmul(
                            ps[:, 0:DG],
                            xc[:, kc, ti * 128:(ti + 1) * 128],
                            wv_sb[:, kc, :],
                            start=(kc == 0),
                            stop=(kc == KC - 1),
                        )
                    for h in range(HG):
                        nc.vector.tensor_copy(
                            out=V[:, tt, h, 0:D], in_=ps[:, h * D:(h + 1) * D])

            # ---- phase 2: attention (scores transposed, head pairs) ----
            # OT2[p, hm, q]: partition p = 64*j + d for head h = 2*hm + j.
            # This matches wo_sb's row layout so fc_out contracts K=128/pair.
            OT2 = otpool.tile([128, 2, SEQ], MD)
            QC = 1024  # q-chunk: one [128, QC] psum = 2 banks, one exp inst

            for hm in range(2):
                for qc in range(SEQ // QC):
                    qs = slice(qc * QC, (qc + 1) * QC)
                    po = [popool.tile([D + 1, QC], F32, name="po", tag="po")
                          for _ in range(2)]
                    for m in range(SEQ // 128):
                        es = []
                        for j in range(2):  # paired heads -> concurrent MMs
                            ps = pspool.tile([128, QC], F32)
                            for ha in range(QC // 512):
                                nc.tensor.matmul(
                                    ps[:, ha * 512:(ha + 1) * 512],
                                    KT[j * D:(j + 1) * D, hm,
                                          m * 128:(m + 1) * 128],
                                    QT[j * D:(j + 1) * D, hm,
                                          qc * QC + ha * 512:
                                          qc * QC + (ha + 1) * 512],
                                    start=True,
                                    stop=True,
                                )
                            e = epool.tile([128, QC], MD)
                            nc.scalar.activation(
                                out=e, in_=ps,
                                func=mybir.ActivationFunctionType.Exp,
                                scale=1.0 / np.sqrt(D),
                            )
                            es.append(e)
                        for j in range(2):
                            for ha in range(QC // 512):
                                nc.tensor.matmul(
                                    po[j][:, ha * 512:(ha + 1) * 512],
                                    V[:, m, 2 * hm + j, :],
                                    es[j][:, ha * 512:(ha + 1) * 512],
                                    start=(m == 0),
                                    stop=(m == SEQ // 128 - 1),
                                )
                    for j in range(2):
                        h = 2 * hm + j
                        st = stpool.tile([D + 1, QC], F32)
                        nc.vector.tensor_copy(out=st, in_=po[j])
                        nc.sync.dma_start(
                            out=OT2[j * D:(j + 1) * D, hm, qs],
                            in_=st[0:D, :].bitcast(MD))
                        nc.sync.dma_start(
                            out=den_dram[h:h + 1, qs], in_=st[D:D + 1, :])

            # reciprocal, reshaped to use all 128 partitions (free dim 64)
            rsm = denpool.tile([128, HG * SEQ // 128], F32)
            den_r = den_dram.rearrange("h (a b) -> (h a) b", a=32)
            rden_r = rden_dram.rearrange("h (a b) -> (h a) b", a=32)
            nc.sync.dma_start(out=rsm, in_=den_r)
            nc.vector.reciprocal(out=rsm, in_=rsm)
            nc.sync.dma_start(out=rden_r, in_=rsm)

            # normalize O^T rows by 1/denominator (broadcast across partitions)
            for hm in range(2):
                rb = rbcpool.tile([128, SEQ], F32)
                for j in range(2):
                    nc.sync.dma_start(
                        out=rb[j * D:(j + 1) * D, :],
                        in_=rden_dram[2 * hm + j:2 * hm + j + 1, :]
                        .to_broadcast((D, SEQ)))
                nc.vector.tensor_mul(OT2[:, hm, :], OT2[:, hm, :], rb)

            # ---- phase 3: partial fc_out  y = sum_h O_h @ Wo_h (K=128/pair) ----
            for tt in range(SEQ // 128):
                for nch in range(EMBED // 512):
                    ps = pspool.tile([128, QC], F32)
                    for hm in range(2):
                        nc.tensor.matmul(
                            ps[:, 0:512],
                            OT2[:, hm, tt * 128:(tt + 1) * 128],
                            wo_sb[:, hm, nch * 512:(nch + 1) * 512],
                            start=(hm == 0),
                            stop=(hm == 1),
                        )
                    ys = ypool.tile([128, 512], F32)
                    nc.vector.tensor_copy(out=ys, in_=ps[:, 0:512])
                    nc.sync.dma_start(
                        out=y[tt * 128:(tt + 1) * 128, nch * 512:(nch + 1) * 512],
                        in_=ys)

    nc.compile()
    return nc


def shard_inputs(x, Wv, Wk, Wq, Wo):
    """Build the 8 per-core input maps."""
    in_maps = []
    for c in range(NCORES):
        n, g = divmod(c, GROUPS)
        cols = slice(g * DG, (g + 1) * DG)
        in_maps.append({
            "xT": np.ascontiguousarray(np.asarray(x[n], dtype=np.float32).T),
            "wq": np.ascontiguousarray(np.asarray(Wq, np.float32)[:, cols]),
            "wk": np.ascontiguousarray(np.asarray(Wk, np.float32)[:, cols]),
            "wv": np.ascontiguousarray(np.asarray(Wv, np.float32)[:, cols]),
            "wo": np.ascontiguousarray(np.asarray(Wo, np.float32)[cols, :]),
        })
    return in_maps


def kernel(x, Wv, Wk, Wq, Wo, bo):
    global LAST_RESULTS
    x = np.asarray(x, np.float32)
    in_maps = shard_inputs(x, Wv, Wk, Wq, Wo)

    if "nc" not in _CACHED_NC:
        _CACHED_NC["nc"] = build_nc()
    nc = _CACHED_NC["nc"]

    trace = os.environ.get("MHA_TRACE", "0") == "1"
    res = bass_utils.run_bass_kernel_spmd(
        nc, in_maps, core_ids=list(range(NCORES)), trace=trace)
    LAST_RESULTS = res

    bo = np.asarray(bo, np.float32)
    out = np.empty((NB, SEQ, EMBED), np.float32)
    for n in range(NB):
        acc = res.results[n * GROUPS]["y"].astype(np.float32).copy()
        for g in range(1, GROUPS):
            acc += res.results[n * GROUPS + g]["y"]
        out[n] = acc + bo[None, :]
    return out


# revision 24
# speedup vs baseline: 2.6860x; 1.2499x over previous
"""Multi-head self-attention Trainium2 kernel (8-core SPMD, full IO).

Problem: x:(2,2048,1024) f32; Wq/Wk/Wv/Wo:(1024,1024); bo:(1024,)
  out = softmax((xWq)(xWk)^T / 8) (xWv) reshaped @ Wo + bo

Sharding: data parallel on batch N=2 x tensor parallel on 16 heads in
4 groups of 4 heads.  Core c handles batch c//4, heads [4*(c%4), 4*(c%4)+4).
Each core computes a partial fc_out product (2048,1024); the host sums the
4 head-group partials per batch and adds the bias.

On-chip layout (per core):
  xT   (1024,2048)  x[n]^T, embed on partitions (8 chunks of 128)
  Q^T/K^T stored as [128, 2, 2048] (dims-chunk on partitions, tokens free)
  V    stored as [128(tokens), 16, 4, 65]; col 64 = ones (denominator trick)
  scores are computed TRANSPOSED: S^T[k,q] so that exp runs on ACT and the
  softmax denominator falls out of the ones-column of V during the O^T
  accumulation (row 64 of the [65,512] psum).  No max subtraction: scores
  are ~N(0,1), bounded well inside fp32 exp range (as in the reference,
  which subtracts max only for stability, not value).
"""

import os

import numpy as np

import concourse.bass as bass
import concourse.tile as tile
from concourse import bacc, mybir
from concourse import bass_utils

F32 = mybir.dt.float32

EMBED = 1024
SEQ = 2048
NB = 2  # batch
HEADS = 16
D = 64  # head dim
NCORES = 8
GROUPS = 4  # head groups (tensor parallel)
HG = HEADS // GROUPS  # heads per core = 4
DG = HG * D  # dims per core = 256

# matmul operand dtype:
#   float32  - exact, 1/4 PE rate
#   float32r - tf32-class (~3e-4 rel), ~2 cyc/row (4-byte stream bound)
#   bfloat16 - ~5e-3 rel, full PE rate, half DMA/SBUF footprint
_MM_DTYPE_NAME = os.environ.get("MHA_MM_DTYPE", "bfloat16")
MM_DTYPE = getattr(mybir.dt, _MM_DTYPE_NAME)
BF16 = mybir.dt.bfloat16

# set by run_cores(); test.py reads exec_time_ns from here
LAST_RESULTS = None
_CACHED_NC = {}


MD = MM_DTYPE  # dtype of matmul-feeding tiles
# DRAM dtype of the big inputs: bf16 inputs are converted host-side (DMA
# cannot cast); f32r shares fp32 bits so DRAM stays f32 + bitcast at DMA.
IN_DT = BF16 if MM_DTYPE == BF16 else F32
IN_NP = None  # numpy dtype for host conversion, set below


def _in_cast(ap):
    """DRAM-side view of an input AP in the matmul dtype."""
    return ap if MD in (F32, BF16) else ap.bitcast(MD)


def build_nc():
    nc = bacc.Bacc("TRN2", target_bir_lowering=False, debug=False,
                   num_devices=NCORES)

    xT = nc.dram_tensor("xT", (EMBED, SEQ), IN_DT, kind="ExternalInput").ap()
    wq = nc.dram_tensor("wq", (EMBED, DG), IN_DT, kind="ExternalInput").ap()
    wk = nc.dram_tensor("wk", (EMBED, DG), IN_DT, kind="ExternalInput").ap()
    wv = nc.dram_tensor("wv", (EMBED, DG), IN_DT, kind="ExternalInput").ap()
    wo = nc.dram_tensor("wo", (DG, EMBED), IN_DT, kind="ExternalInput").ap()
    y = nc.dram_tensor("y", (SEQ, EMBED), F32, kind="ExternalOutput").ap()
    # DRAM bounce buffers for the softmax denominators: SBUF sources can't be
    # partition-broadcast by DMA, DRAM sources can.
    # row qc*HG + h holds head h's denominators for q-chunk qc
    den_dram = nc.dram_tensor("den_scratch", (2 * HG, 1024), F32).ap()
    rden_dram = nc.dram_tensor("rden_scratch", (2 * HG, 1024), F32).ap()

    KC = EMBED // 128  # 8 contraction chunks for projections

    with tile.TileContext(nc) as tc:
        with (
            tc.tile_pool(name="weights", bufs=1) as wpool,
            tc.tile_pool(name="qk", bufs=1) as qkpool,
            tc.tile_pool(name="vpool", bufs=1) as vpool,
            tc.tile_pool(name="otpool", bufs=1) as otpool,
            tc.tile_pool(name="xchunk", bufs=2) as xpool,
            tc.tile_pool(name="epool", bufs=4) as epool,
            tc.tile_pool(name="stage", bufs=2) as stpool,
            tc.tile_pool(name="den", bufs=1) as denpool,
            tc.tile_pool(name="rbc", bufs=2) as rbcpool,
            tc.tile_pool(name="ystage", bufs=3) as ypool,
            tc.tile_pool(name="psum", bufs=2, space="PSUM") as pspool,
            tc.tile_pool(name="psum_o", bufs=2, space="PSUM") as popool,
        ):
            # ---- load weights ----
            wq_sb = wpool.tile([128, KC, DG], MD)
            wk_sb = wpool.tile([128, KC, DG], MD)
            wv_sb = wpool.tile([128, KC, DG], MD)
            wo_sb = wpool.tile([128, DG // 128, EMBED], MD)
            nc.sync.dma_start(out=wq_sb, in_=_in_cast(wq).rearrange("(c p) n -> p c n", p=128))
            nc.sync.dma_start(out=wk_sb, in_=_in_cast(wk).rearrange("(c p) n -> p c n", p=128))
            nc.sync.dma_start(out=wv_sb, in_=_in_cast(wv).rearrange("(c p) n -> p c n", p=128))
            nc.sync.dma_start(out=wo_sb, in_=_in_cast(wo).rearrange("(c p) n -> p c n", p=128))

            QT = qkpool.tile([128, 2, SEQ], MD)  # [dim-chunk part, mt, tokens]
            KT = qkpool.tile([128, 2, SEQ], MD)
            V = vpool.tile([128, 16, HG, D + 1], MD)  # [tok part, tok-tile, head, d+1]
            ones_col = V[:, :, :, D:D + 1]
            nc.vector.memset(
                ones_col.bitcast(F32) if MD == mybir.dt.float32r else ones_col,
                1.0)

            xTr = _in_cast(xT).rearrange("(c p) s -> p c s", p=128)

            # ---- phase 1: projections, one 256-token chunk at a time ----
            TCH = 512
            for tcb in range(SEQ // TCH):
                xc = xpool.tile([128, KC, TCH], MD)
                nc.sync.dma_start(out=xc, in_=xTr[:, :, tcb * TCH:(tcb + 1) * TCH])

                # Q^T and K^T chunks: [dims 128, tokens TCH]
                for wsb, dst in ((wq_sb, QT), (wk_sb, KT)):
                    for mt in range(2):
                        ps = pspool.tile([128, 512], F32)
                        for kc in range(KC):
                            nc.tensor.matmul(
                                ps[:, 0:TCH],
                                wsb[:, kc, mt * 128:(mt + 1) * 128],
                                xc[:, kc, :],
                                start=(kc == 0),
                                stop=(kc == KC - 1),
                            )
                        nc.vector.tensor_copy(
                            out=dst[:, mt, tcb * TCH:(tcb + 1) * TCH],
                            in_=ps[:, 0:TCH])

                # V chunks: [tokens 128, dims 256]
                for ti in range(TCH // 128):
                    tt = tcb * (TCH // 128) + ti
                    ps = pspool.tile([128, 512], F32)
                    for kc in range(KC):
                        nc.tensor.matmul(
                            ps[:, 0:DG],
                            xc[:, kc, ti * 128:(ti + 1) * 128],
                            wv_sb[:, kc, :],
                            start=(kc == 0),
                            stop=(kc == KC - 1),
                        )
                    for h in range(HG):
                        nc.vector.tensor_copy(
                            out=V[:, tt, h, 0:D], in_=ps[:, h * D:(h + 1) * D])

            # ---- phase 2: attention (scores transposed, head pairs) ----
            # OT2[p, hm, q]: partition p = 64*j + d for head h = 2*hm + j.
            # This matches wo_sb's row layout so fc_out contracts K=128/pair.
            OT2 = otpool.tile([128, 2, SEQ], MD)
            QC = 1024  # q-chunk: one [128, QC] psum = 2 banks, one exp inst

            for qc in range(SEQ // QC):
                qs = slice(qc * QC, (qc + 1) * QC)
                for hm in range(2):
                    po = [popool.tile([D + 1, QC], F32, name="po", tag="po")
                          for _ in range(2)]
                    for m in range(SEQ // 128):
                        es = []
                        for j in range(2):  # paired heads -> concurrent MMs
                            ps = pspool.tile([128, QC], F32)
                            for ha in range(QC // 512):
                                nc.tensor.matmul(
                                    ps[:, ha * 512:(ha + 1) * 512],
                                    KT[j * D:(j + 1) * D, hm,
                                       m * 128:(m + 1) * 128],
                                    QT[j * D:(j + 1) * D, hm,
                                       qc * QC + ha * 512:
                                       qc * QC + (ha + 1) * 512],
                                    start=True,
                                    stop=True,
                                )
                            e = epool.tile([128, QC], MD)
                            nc.scalar.activation(
                                out=e, in_=ps,
                                func=mybir.ActivationFunctionType.Exp,
                                scale=1.0 / np.sqrt(D),
                            )
                            es.append(e)
                        for j in range(2):
                            for ha in range(QC // 512):
                                nc.tensor.matmul(
                                    po[j][:, ha * 512:(ha + 1) * 512],
                                    V[:, m, 2 * hm + j, :],
                                    es[j][:, ha * 512:(ha + 1) * 512],
                                    start=(m == 0),
                                    stop=(m == SEQ // 128 - 1),
                                )
                    for j in range(2):
                        h = 2 * hm + j
                        st = stpool.tile([D + 1, QC], F32)
                        nc.vector.tensor_copy(out=st, in_=po[j])
                        ot_dst = OT2[j * D:(j + 1) * D, hm, qs]
                        if MD == BF16:
                            nc.gpsimd.dma_start(out=ot_dst, in_=st[0:D, :])
                        elif MD == F32:
                            nc.sync.dma_start(out=ot_dst, in_=st[0:D, :])
                        else:
                            nc.sync.dma_start(
                                out=ot_dst, in_=st[0:D, :].bitcast(MD))
                        nc.sync.dma_start(
                            out=den_dram[qc * HG + h:qc * HG + h + 1, :],
                            in_=st[D:D + 1, :])

                # per-chunk normalize + fc_out (overlaps next chunk's attention)
                rsm = denpool.tile([128, HG * QC // 128], F32, name="rsm",
                                   tag="rsm")
                rows = slice(qc * HG, (qc + 1) * HG)
                den_r = den_dram[rows, :].rearrange("h (a b) -> (h a) b", a=32)
                rden_r = rden_dram[rows, :].rearrange("h (a b) -> (h a) b", a=32)
                nc.sync.dma_start(out=rsm, in_=den_r)
                nc.vector.reciprocal(out=rsm, in_=rsm)
                nc.sync.dma_start(out=rden_r, in_=rsm)

                for hm in range(2):
                    rb = rbcpool.tile([128, QC], F32, name="rb", tag="rb")
                    for j in range(2):
                        nc.sync.dma_start(
                            out=rb[j * D:(j + 1) * D, :],
                            in_=rden_dram[qc * HG + 2 * hm + j:
                                          qc * HG + 2 * hm + j + 1, :]
                            .to_broadcast((D, QC)))
                    nc.vector.tensor_mul(OT2[:, hm, qs], OT2[:, hm, qs], rb)

                for tt in range(qc * QC // 128, (qc + 1) * QC // 128):
                    for nch in range(EMBED // 512):
                        ps = pspool.tile([128, QC], F32)
                        for hm in range(2):
                            nc.tensor.matmul(
                                ps[:, 0:512],
                                OT2[:, hm, tt * 128:(tt + 1) * 128],
                                wo_sb[:, hm, nch * 512:(nch + 1) * 512],
                                start=(hm == 0),
                                stop=(hm == 1),
                            )
                        ys = ypool.tile([128, 512], F32)
                        if tt % 2 == 0:
                            nc.vector.tensor_copy(out=ys, in_=ps[:, 0:512])
                        else:
                            nc.scalar.copy(out=ys, in_=ps[:, 0:512])
                        nc.sync.dma_start(
                            out=y[tt * 128:(tt + 1) * 128,
                                  nch * 512:(nch + 1) * 512],
                            in_=ys)

    nc.compile()
    return nc


def shard_inputs(x, Wv, Wk, Wq, Wo):
    """Build the 8 per-core input maps."""
    in_maps = []
    for c in range(NCORES):
        n, g = divmod(c, GROUPS)
        cols = slice(g * DG, (g + 1) * DG)
        wire = np.float32
        if MM_DTYPE == BF16:
            import ml_dtypes
            wire = ml_dtypes.bfloat16
        in_maps.append({
            "xT": np.ascontiguousarray(np.asarray(x[n], np.float32).T).astype(wire),
            "wq": np.ascontiguousarray(np.asarray(Wq, np.float32)[:, cols]).astype(wire),
            "wk": np.ascontiguousarray(np.asarray(Wk, np.float32)[:, cols]).astype(wire),
            "wv": np.ascontiguousarray(np.asarray(Wv, np.float32)[:, cols]).astype(wire),
            "wo": np.ascontiguousarray(np.asarray(Wo, np.float32)[cols, :]).astype(wire),
        })
    return in_maps


def kernel(x, Wv, Wk, Wq, Wo, bo):
    global LAST_RESULTS
    x = np.asarray(x, np.float32)
    in_maps = shard_inputs(x, Wv, Wk, Wq, Wo)

    if "nc" not in _CACHED_NC:
        _CACHED_NC["nc"] = build_nc()
    nc = _CACHED_NC["nc"]

    trace = os.environ.get("MHA_TRACE", "0") == "1"
    res = bass_utils.run_bass_kernel_spmd(
        nc, in_maps, core_ids=list(range(NCORES)), trace=trace)
    LAST_RESULTS = res

    bo = np.asarray(bo, np.float32)
    out = np.empty((NB, SEQ, EMBED), np.float32)
    for n in range(NB):
        acc = res.results[n * GROUPS]["y"].astype(np.float32).copy()
        for g in range(1, GROUPS):
            acc += res.results[n * GROUPS + g]["y"]
        out[n] = acc + bo[None, :]
    return out


# revision 26
# speedup vs baseline: 3.2112x; 1.1955x over previous
"""Multi-head self-attention Trainium2 kernel (8-core SPMD, full IO).

Problem: x:(2,2048,1024) f32; Wq/Wk/Wv/Wo:(1024,1024); bo:(1024,)
  out = softmax((xWq)(xWk)^T / 8) (xWv) reshaped @ Wo + bo

Sharding: data parallel on batch N=2 x tensor parallel on 16 heads in
4 groups of 4 heads.  Core c handles batch c//4, heads [4*(c%4), 4*(c%4)+4).
Each core computes a partial fc_out product (2048,1024); the host sums the
4 head-group partials per batch and adds the bias.

On-chip layout (per core):
  xT   (1024,2048)  x[n]^T, embed on partitions (8 chunks of 128)
  Q^T/K^T stored as [128, 2, 2048] (dims-chunk on partitions, tokens free)
  V    stored as [128(tokens), 16, 4, 65]; col 64 = ones (denominator trick)
  scores are computed TRANSPOSED: S^T[k,q] so that exp runs on ACT and the
  softmax denominator falls out of the ones-column of V during the O^T
  accumulation (row 64 of the [65,512] psum).  No max subtraction: scores
  are ~N(0,1), bounded well inside fp32 exp range (as in the reference,
  which subtracts max only for stability, not value).
"""

import os

import numpy as np

import concourse.bass as bass
import concourse.tile as tile
from concourse import bacc, mybir
from concourse import bass_utils

F32 = mybir.dt.float32

EMBED = 1024
SEQ = 2048
NB = 2  # batch
HEADS = 16
D = 64  # head dim
NCORES = 8
GROUPS = 4  # head groups (tensor parallel)
HG = HEADS // GROUPS  # heads per core = 4
DG = HG * D  # dims per core = 256

# matmul operand dtype:
#   float32  - exact, 1/4 PE rate
#   float32r - tf32-class (~3e-4 rel), ~2 cyc/row (4-byte stream bound)
#   bfloat16 - ~5e-3 rel, full PE rate, half DMA/SBUF footprint
_MM_DTYPE_NAME = os.environ.get("MHA_MM_DTYPE", "bfloat16")
MM_DTYPE = getattr(mybir.dt, _MM_DTYPE_NAME)
BF16 = mybir.dt.bfloat16

# set by run_cores(); test.py reads exec_time_ns from here
LAST_RESULTS = None
_CACHED_NC = {}


MD = MM_DTYPE  # dtype of matmul-feeding tiles
# DRAM dtype of the big inputs: bf16 inputs are converted host-side (DMA
# cannot cast); f32r shares fp32 bits so DRAM stays f32 + bitcast at DMA.
IN_DT = BF16 if MM_DTYPE == BF16 else F32
IN_NP = None  # numpy dtype for host conversion, set below


def _in_cast(ap):
    """DRAM-side view of an input AP in the matmul dtype."""
    return ap if MD in (F32, BF16) else ap.bitcast(MD)


def build_nc():
    nc = bacc.Bacc("TRN2", target_bir_lowering=False, debug=False,
                   num_devices=NCORES)

    xT = nc.dram_tensor("xT", (EMBED, SEQ), IN_DT, kind="ExternalInput").ap()
    wq = nc.dram_tensor("wq", (EMBED, DG), IN_DT, kind="ExternalInput").ap()
    wk = nc.dram_tensor("wk", (EMBED, DG), IN_DT, kind="ExternalInput").ap()
    wv = nc.dram_tensor("wv", (EMBED, DG), IN_DT, kind="ExternalInput").ap()
    wo = nc.dram_tensor("wo", (DG, EMBED), IN_DT, kind="ExternalInput").ap()
    y = nc.dram_tensor("y", (SEQ, EMBED), F32, kind="ExternalOutput").ap()
    # DRAM bounce buffers for the softmax denominators: SBUF sources can't be
    # partition-broadcast by DMA, DRAM sources can.
    den_dram = nc.dram_tensor("den_scratch", (HG, SEQ), F32).ap()
    rden_dram = nc.dram_tensor("rden_scratch", (HG, SEQ), F32).ap()

    KC = EMBED // 128  # 8 contraction chunks for projections

    with tile.TileContext(nc) as tc:
        with (
            tc.tile_pool(name="weights", bufs=1) as wpool,
            tc.tile_pool(name="qk", bufs=1) as qkpool,
            tc.tile_pool(name="vpool", bufs=1) as vpool,
            tc.tile_pool(name="otpool", bufs=1) as otpool,
            tc.tile_pool(name="xchunk", bufs=2) as xpool,
            tc.tile_pool(name="epool", bufs=6) as epool,
            tc.tile_pool(name="stage", bufs=4) as stpool,
            tc.tile_pool(name="den", bufs=1) as denpool,
            tc.tile_pool(name="rbc", bufs=2) as rbcpool,
            tc.tile_pool(name="ystage", bufs=3) as ypool,
            tc.tile_pool(name="psum", bufs=2, space="PSUM") as pspool,
            tc.tile_pool(name="psum_o", bufs=2, space="PSUM") as popool,
        ):
            # ---- load weights ----
            wq_sb = wpool.tile([128, KC, DG], MD)
            wk_sb = wpool.tile([128, KC, DG], MD)
            wv_sb = wpool.tile([128, KC, DG], MD)
            wo_sb = wpool.tile([128, DG // 128, EMBED], MD)
            nc.sync.dma_start(out=wq_sb, in_=_in_cast(wq).rearrange("(c p) n -> p c n", p=128))
            nc.sync.dma_start(out=wk_sb, in_=_in_cast(wk).rearrange("(c p) n -> p c n", p=128))
            nc.sync.dma_start(out=wv_sb, in_=_in_cast(wv).rearrange("(c p) n -> p c n", p=128))
            nc.sync.dma_start(out=wo_sb, in_=_in_cast(wo).rearrange("(c p) n -> p c n", p=128))

            QT = qkpool.tile([128, 2, SEQ], MD)  # [dim-chunk part, mt, tokens]
            KT = qkpool.tile([128, 2, SEQ], MD)
            V = vpool.tile([128, 16, HG, D + 1], MD)  # [tok part, tok-tile, head, d+1]
            ones_col = V[:, :, :, D:D + 1]
            nc.vector.memset(
                ones_col.bitcast(F32) if MD == mybir.dt.float32r else ones_col,
                1.0)

            xTr = _in_cast(xT).rearrange("(c p) s -> p c s", p=128)

            # ---- phase 1: projections, one 256-token chunk at a time ----
            TCH = 512
            for tcb in range(SEQ // TCH):
                xc = xpool.tile([128, KC, TCH], MD)
                nc.sync.dma_start(out=xc, in_=xTr[:, :, tcb * TCH:(tcb + 1) * TCH])

                # Q^T and K^T chunks: [dims 128, tokens TCH]
                for wsb, dst in ((wq_sb, QT), (wk_sb, KT)):
                    for mt in range(2):
                        ps = pspool.tile([128, 512], F32)
                        for kc in range(KC):
                            nc.tensor.matmul(
                                ps[:, 0:TCH],
                                wsb[:, kc, mt * 128:(mt + 1) * 128],
                                xc[:, kc, :],
                                start=(kc == 0),
                                stop=(kc == KC - 1),
                            )
                        nc.vector.tensor_copy(
                            out=dst[:, mt, tcb * TCH:(tcb + 1) * TCH],
                            in_=ps[:, 0:TCH])

                # V chunks: [tokens 128, dims 256]
                for ti in range(TCH // 128):
                    tt = tcb * (TCH // 128) + ti
                    ps = pspool.tile([128, 512], F32)
                    for kc in range(KC):
                        nc.tensor.matmul(
                            ps[:, 0:DG],
                            xc[:, kc, ti * 128:(ti + 1) * 128],
                            wv_sb[:, kc, :],
                            start=(kc == 0),
                            stop=(kc == KC - 1),
                        )
                    nc.vector.tensor_copy(
                        out=V[:, tt, :, 0:D],
                        in_=ps[:, 0:DG].rearrange("p (h d) -> p h d", h=HG))

            # ---- phase 2: attention (scores transposed, head pairs) ----
            # OT2[p, hm, q]: partition p = 64*j + d for head h = 2*hm + j.
            # This matches wo_sb's row layout so fc_out contracts K=128/pair.
            OT2 = otpool.tile([128, 2, SEQ], MD)
            QC = 1024  # q-chunk: one [128, QC] psum = 2 banks, one exp inst

            for hm in range(2):
                for qc in range(SEQ // QC):
                    qs = slice(qc * QC, (qc + 1) * QC)
                    po = [popool.tile([D + 1, QC], F32, name="po", tag="po")
                          for _ in range(2)]
                    for m in range(SEQ // 128):
                        es = []
                        for j in range(2):  # paired heads -> concurrent MMs
                            ps = pspool.tile([128, QC], F32)
                            for ha in range(QC // 512):
                                nc.tensor.matmul(
                                    ps[:, ha * 512:(ha + 1) * 512],
                                    KT[j * D:(j + 1) * D, hm,
                                          m * 128:(m + 1) * 128],
                                    QT[j * D:(j + 1) * D, hm,
                                          qc * QC + ha * 512:
                                          qc * QC + (ha + 1) * 512],
                                    start=True,
                                    stop=True,
                                )
                            e = epool.tile([128, QC], MD)
                            nc.scalar.activation(
                                out=e, in_=ps,
                                func=mybir.ActivationFunctionType.Exp,
                                scale=1.0 / np.sqrt(D),
                            )
                            es.append(e)
                        for j in range(2):
                            for ha in range(QC // 512):
                                nc.tensor.matmul(
                                    po[j][:, ha * 512:(ha + 1) * 512],
                                    V[:, m, 2 * hm + j, :],
                                    es[j][:, ha * 512:(ha + 1) * 512],
                                    start=(m == 0),
                                    stop=(m == SEQ // 128 - 1),
                                )
                    for j in range(2):
                        h = 2 * hm + j
                        st = stpool.tile([D + 1, QC], F32)
                        nc.vector.tensor_copy(out=st, in_=po[j])
                        ot_dst = OT2[j * D:(j + 1) * D, hm, qs]
                        if MD == BF16:
                            nc.gpsimd.dma_start(out=ot_dst, in_=st[0:D, :])
                        elif MD == F32:
                            nc.sync.dma_start(out=ot_dst, in_=st[0:D, :])
                        else:
                            nc.sync.dma_start(
                                out=ot_dst, in_=st[0:D, :].bitcast(MD))
                        nc.sync.dma_start(
                            out=den_dram[h:h + 1, qs], in_=st[D:D + 1, :])

            # reciprocal, reshaped to use all 128 partitions (free dim 64)
            rsm = denpool.tile([128, HG * SEQ // 128], F32)
            den_r = den_dram.rearrange("h (a b) -> (h a) b", a=32)
            rden_r = rden_dram.rearrange("h (a b) -> (h a) b", a=32)
            nc.sync.dma_start(out=rsm, in_=den_r)
            nc.vector.reciprocal(out=rsm, in_=rsm)
            nc.sync.dma_start(out=rden_r, in_=rsm)

            # normalize O^T rows by 1/denominator (broadcast across partitions)
            for hm in range(2):
                rb = rbcpool.tile([128, SEQ], F32)
                for j in range(2):
                    nc.sync.dma_start(
                        out=rb[j * D:(j + 1) * D, :],
                        in_=rden_dram[2 * hm + j:2 * hm + j + 1, :]
                        .to_broadcast((D, SEQ)))
                nc.vector.tensor_mul(OT2[:, hm, :], OT2[:, hm, :], rb)

            # ---- phase 3: partial fc_out  y = sum_h O_h @ Wo_h (K=128/pair) ----
            for tt in range(SEQ // 128):
                for nch in range(EMBED // 512):
                    ps = pspool.tile([128, QC], F32)
                    for hm in range(2):
                        nc.tensor.matmul(
                            ps[:, 0:512],
                            OT2[:, hm, tt * 128:(tt + 1) * 128],
                            wo_sb[:, hm, nch * 512:(nch + 1) * 512],
                            start=(hm == 0),
                            stop=(hm == 1),
                        )
                    ys = ypool.tile([128, 512], F32)
                    nc.vector.tensor_copy(out=ys, in_=ps[:, 0:512])
                    nc.sync.dma_start(
                        out=y[tt * 128:(tt + 1) * 128, nch * 512:(nch + 1) * 512],
                        in_=ys)

    nc.compile()
    return nc


def shard_inputs(x, Wv, Wk, Wq, Wo):
    """Build the 8 per-core input maps."""
    in_maps = []
    for c in range(NCORES):
        n, g = divmod(c, GROUPS)
        cols = slice(g * DG, (g + 1) * DG)
        wire = np.float32
        if MM_DTYPE == BF16:
            import ml_dtypes
            wire = ml_dtypes.bfloat16
        in_maps.append({
            "xT": np.ascontiguousarray(np.asarray(x[n], np.float32).T).astype(wire),
            "wq": np.ascontiguousarray(np.asarray(Wq, np.float32)[:, cols]).astype(wire),
            "wk": np.ascontiguousarray(np.asarray(Wk, np.float32)[:, cols]).astype(wire),
            "wv": np.ascontiguousarray(np.asarray(Wv, np.float32)[:, cols]).astype(wire),
            "wo": np.ascontiguousarray(np.asarray(Wo, np.float32)[cols, :]).astype(wire),
        })
    return in_maps


def kernel(x, Wv, Wk, Wq, Wo, bo):
    global LAST_RESULTS
    x = np.asarray(x, np.float32)
    in_maps = shard_inputs(x, Wv, Wk, Wq, Wo)

    if "nc" not in _CACHED_NC:
        _CACHED_NC["nc"] = build_nc()
    nc = _CACHED_NC["nc"]

    trace = os.environ.get("MHA_TRACE", "0") == "1"
    res = bass_utils.run_bass_kernel_spmd(
        nc, in_maps, core_ids=list(range(NCORES)), trace=trace)
    LAST_RESULTS = res

    bo = np.asarray(bo, np.float32)
    out = np.empty((NB, SEQ, EMBED), np.float32)
    for n in range(NB):
        acc = res.results[n * GROUPS]["y"].astype(np.float32).copy()
        for g in range(1, GROUPS):
            acc += res.results[n * GROUPS + g]["y"]
        out[n] = acc + bo[None, :]
    return out


# revision 27
# speedup vs baseline: 3.2723x; 1.0190x over previous
"""Multi-head self-attention Trainium2 kernel (8-core SPMD, full IO).

Problem: x:(2,2048,1024) f32; Wq/Wk/Wv/Wo:(1024,1024); bo:(1024,)
  out = softmax((xWq)(xWk)^T / 8) (xWv) reshaped @ Wo + bo

Sharding: data parallel on batch N=2 x tensor parallel on 16 heads in
4 groups of 4 heads.  Core c handles batch c//4, heads [4*(c%4), 4*(c%4)+4).
Each core computes a partial fc_out product (2048,1024); the host sums the
4 head-group partials per batch and adds the bias.

On-chip layout (per core):
  xT   (1024,2048)  x[n]^T, embed on partitions (8 chunks of 128)
  Q^T/K^T stored as [128, 2, 2048] (dims-chunk on partitions, tokens free)
  V    stored as [128(tokens), 16, 4, 65]; col 64 = ones (denominator trick)
  scores are computed TRANSPOSED: S^T[k,q] so that exp runs on ACT and the
  softmax denominator falls out of the ones-column of V during the O^T
  accumulation (row 64 of the [65,512] psum).  No max subtraction: scores
  are ~N(0,1), bounded well inside fp32 exp range (as in the reference,
  which subtracts max only for stability, not value).
"""

import os

import numpy as np

import concourse.bass as bass
import concourse.tile as tile
from concourse import bacc, mybir
from concourse import bass_utils

F32 = mybir.dt.float32

EMBED = 1024
SEQ = 2048
NB = 2  # batch
HEADS = 16
D = 64  # head dim
NCORES = 8
GROUPS = 4  # head groups (tensor parallel)
HG = HEADS // GROUPS  # heads per core = 4
DG = HG * D  # dims per core = 256

# matmul operand dtype:
#   float32  - exact, 1/4 PE rate
#   float32r - tf32-class (~3e-4 rel), ~2 cyc/row (4-byte stream bound)
#   bfloat16 - ~5e-3 rel, full PE rate, half DMA/SBUF footprint
_MM_DTYPE_NAME = os.environ.get("MHA_MM_DTYPE", "bfloat16")
MM_DTYPE = getattr(mybir.dt, _MM_DTYPE_NAME)
BF16 = mybir.dt.bfloat16

# set by run_cores(); test.py reads exec_time_ns from here
LAST_RESULTS = None
_CACHED_NC = {}


MD = MM_DTYPE  # dtype of matmul-feeding tiles
# DRAM dtype of the big inputs: bf16 inputs are converted host-side (DMA
# cannot cast); f32r shares fp32 bits so DRAM stays f32 + bitcast at DMA.
IN_DT = BF16 if MM_DTYPE == BF16 else F32
IN_NP = None  # numpy dtype for host conversion, set below


def _in_cast(ap):
    """DRAM-side view of an input AP in the matmul dtype."""
    return ap if MD in (F32, BF16) else ap.bitcast(MD)


def build_nc():
    nc = bacc.Bacc("TRN2", target_bir_lowering=False, debug=False,
                   num_devices=NCORES)

    xT = nc.dram_tensor("xT", (EMBED, SEQ), IN_DT, kind="ExternalInput").ap()
    wq = nc.dram_tensor("wq", (EMBED, DG), IN_DT, kind="ExternalInput").ap()
    wk = nc.dram_tensor("wk", (EMBED, DG), IN_DT, kind="ExternalInput").ap()
    wv = nc.dram_tensor("wv", (EMBED, DG), IN_DT, kind="ExternalInput").ap()
    wo = nc.dram_tensor("wo", (DG, EMBED), IN_DT, kind="ExternalInput").ap()
    y = nc.dram_tensor("y", (SEQ, EMBED), F32, kind="ExternalOutput").ap()
    # DRAM bounce buffers for the softmax denominators: SBUF sources can't be
    # partition-broadcast by DMA, DRAM sources can.
    den_dram = nc.dram_tensor("den_scratch", (HG, SEQ), F32).ap()
    rden_dram = nc.dram_tensor("rden_scratch", (HG, SEQ), F32).ap()

    KC = EMBED // 128  # 8 contraction chunks for projections

    with tile.TileContext(nc) as tc:
        with (
            tc.tile_pool(name="weights", bufs=1) as wpool,
            tc.tile_pool(name="qk", bufs=1) as qkpool,
            tc.tile_pool(name="vpool", bufs=1) as vpool,
            tc.tile_pool(name="otpool", bufs=1) as otpool,
            tc.tile_pool(name="xchunk", bufs=2) as xpool,
            tc.tile_pool(name="epool", bufs=6) as epool,
            tc.tile_pool(name="stage", bufs=4) as stpool,
            tc.tile_pool(name="den", bufs=1) as denpool,
            tc.tile_pool(name="rbc", bufs=2) as rbcpool,
            tc.tile_pool(name="ystage", bufs=3) as ypool,
            tc.tile_pool(name="psum", bufs=2, space="PSUM") as pspool,
            tc.tile_pool(name="psum_o", bufs=2, space="PSUM") as popool,
        ):
            # ---- load weights ----
            wq_sb = wpool.tile([128, KC, DG], MD)
            wk_sb = wpool.tile([128, KC, DG], MD)
            wv_sb = wpool.tile([128, KC, DG], MD)
            wo_sb = wpool.tile([128, DG // 128, EMBED], MD)
            nc.sync.dma_start(out=wq_sb, in_=_in_cast(wq).rearrange("(c p) n -> p c n", p=128))
            nc.sync.dma_start(out=wk_sb, in_=_in_cast(wk).rearrange("(c p) n -> p c n", p=128))
            nc.sync.dma_start(out=wv_sb, in_=_in_cast(wv).rearrange("(c p) n -> p c n", p=128))
            nc.sync.dma_start(out=wo_sb, in_=_in_cast(wo).rearrange("(c p) n -> p c n", p=128))

            # per-512-token-chunk tiles: finer dependency granularity lets
            # phase-2 attention start as soon as its chunks are projected
            QTs = [qkpool.tile([128, 2, 512], MD, name=f"qt{t}", tag=f"qt{t}")
                   for t in range(4)]
            KTs = [qkpool.tile([128, 2, 512], MD, name=f"kt{t}", tag=f"kt{t}")
                   for t in range(4)]
            Vs = [vpool.tile([128, 4, HG, D + 1], MD, name=f"v{t}", tag=f"v{t}")
                  for t in range(4)]
            for t in range(4):
                ones_col = Vs[t][:, :, :, D:D + 1]
                nc.vector.memset(
                    ones_col.bitcast(F32) if MD == mybir.dt.float32r
                    else ones_col, 1.0)

            xTr = _in_cast(xT).rearrange("(c p) s -> p c s", p=128)

            # ---- phase 1: projections, one 256-token chunk at a time ----
            TCH = 512
            for tcb in range(SEQ // TCH):
                xc = xpool.tile([128, KC, TCH], MD)
                nc.sync.dma_start(out=xc, in_=xTr[:, :, tcb * TCH:(tcb + 1) * TCH])

                # Q^T and K^T chunks: [dims 128, tokens TCH]
                for wsb, dst in ((wq_sb, QTs), (wk_sb, KTs)):
                    for mt in range(2):
                        ps = pspool.tile([128, 512], F32)
                        for kc in range(KC):
                            nc.tensor.matmul(
                                ps[:, 0:TCH],
                                wsb[:, kc, mt * 128:(mt + 1) * 128],
                                xc[:, kc, :],
                                start=(kc == 0),
                                stop=(kc == KC - 1),
                            )
                        nc.vector.tensor_copy(
                            out=dst[tcb][:, mt, :], in_=ps[:, 0:TCH])

                # V chunks: [tokens 128, dims 256]
                for ti in range(TCH // 128):
                    tt = tcb * (TCH // 128) + ti
                    ps = pspool.tile([128, 512], F32)
                    for kc in range(KC):
                        nc.tensor.matmul(
                            ps[:, 0:DG],
                            xc[:, kc, ti * 128:(ti + 1) * 128],
                            wv_sb[:, kc, :],
                            start=(kc == 0),
                            stop=(kc == KC - 1),
                        )
                    nc.vector.tensor_copy(
                        out=Vs[tcb][:, ti, :, 0:D],
                        in_=ps[:, 0:DG].rearrange("p (h d) -> p h d", h=HG))

            # ---- phase 2: attention (scores transposed, head pairs) ----
            # OT2[p, hm, q]: partition p = 64*j + d for head h = 2*hm + j.
            # This matches wo_sb's row layout so fc_out contracts K=128/pair.
            OT2 = otpool.tile([128, 2, SEQ], MD)
            QC = 1024  # q-chunk: one [128, QC] psum = 2 banks, one exp inst

            for hm in range(2):
                for qc in range(SEQ // QC):
                    qs = slice(qc * QC, (qc + 1) * QC)
                    po = [popool.tile([D + 1, QC], F32, name="po", tag="po")
                          for _ in range(2)]
                    for m in range(SEQ // 128):
                        es = []
                        for j in range(2):  # paired heads -> concurrent MMs
                            ps = pspool.tile([128, QC], F32)
                            for ha in range(QC // 512):
                                nc.tensor.matmul(
                                    ps[:, ha * 512:(ha + 1) * 512],
                                    KTs[m // 4][j * D:(j + 1) * D, hm,
                                                (m % 4) * 128:
                                                (m % 4 + 1) * 128],
                                    QTs[2 * qc + ha][j * D:(j + 1) * D, hm, :],
                                    start=True,
                                    stop=True,
                                )
                            e = epool.tile([128, QC], MD)
                            nc.scalar.activation(
                                out=e, in_=ps,
                                func=mybir.ActivationFunctionType.Exp,
                                scale=1.0 / np.sqrt(D),
                            )
                            es.append(e)
                        for j in range(2):
                            for ha in range(QC // 512):
                                nc.tensor.matmul(
                                    po[j][:, ha * 512:(ha + 1) * 512],
                                    Vs[m // 4][:, m % 4, 2 * hm + j, :],
                                    es[j][:, ha * 512:(ha + 1) * 512],
                                    start=(m == 0),
                                    stop=(m == SEQ // 128 - 1),
                                )
                    for j in range(2):
                        h = 2 * hm + j
                        st = stpool.tile([D + 1, QC], F32)
                        nc.vector.tensor_copy(out=st, in_=po[j])
                        ot_dst = OT2[j * D:(j + 1) * D, hm, qs]
                        if MD == BF16:
                            nc.gpsimd.dma_start(out=ot_dst, in_=st[0:D, :])
                        elif MD == F32:
                            nc.sync.dma_start(out=ot_dst, in_=st[0:D, :])
                        else:
                            nc.sync.dma_start(
                                out=ot_dst, in_=st[0:D, :].bitcast(MD))
                        nc.sync.dma_start(
                            out=den_dram[h:h + 1, qs], in_=st[D:D + 1, :])

            # reciprocal, reshaped to use all 128 partitions (free dim 64)
            rsm = denpool.tile([128, HG * SEQ // 128], F32)
            den_r = den_dram.rearrange("h (a b) -> (h a) b", a=32)
            rden_r = rden_dram.rearrange("h (a b) -> (h a) b", a=32)
            nc.sync.dma_start(out=rsm, in_=den_r)
            nc.vector.reciprocal(out=rsm, in_=rsm)
            nc.sync.dma_start(out=rden_r, in_=rsm)

            # normalize O^T rows by 1/denominator (broadcast across partitions)
            for hm in range(2):
                rb = rbcpool.tile([128, SEQ], F32)
                for j in range(2):
                    nc.sync.dma_start(
                        out=rb[j * D:(j + 1) * D, :],
                        in_=rden_dram[2 * hm + j:2 * hm + j + 1, :]
                        .to_broadcast((D, SEQ)))
                nc.vector.tensor_mul(OT2[:, hm, :], OT2[:, hm, :], rb)

            # ---- phase 3: partial fc_out  y = sum_h O_h @ Wo_h (K=128/pair) ----
            for tt in range(SEQ // 128):
                for nch in range(EMBED // 512):
                    ps = pspool.tile([128, QC], F32)
                    for hm in range(2):
                        nc.tensor.matmul(
                            ps[:, 0:512],
                            OT2[:, hm, tt * 128:(tt + 1) * 128],
                            wo_sb[:, hm, nch * 512:(nch + 1) * 512],
                            start=(hm == 0),
                            stop=(hm == 1),
                        )
                    ys = ypool.tile([128, 512], F32)
                    if (tt + nch) % 2 == 0:
                        nc.vector.tensor_copy(out=ys, in_=ps[:, 0:512])
                    else:
                        nc.scalar.copy(out=ys, in_=ps[:, 0:512])
                    nc.sync.dma_start(
                        out=y[tt * 128:(tt + 1) * 128, nch * 512:(nch + 1) * 512],
                        in_=ys)

    nc.compile()
    return nc


def shard_inputs(x, Wv, Wk, Wq, Wo):
    """Build the 8 per-core input maps."""
    in_maps = []
    for c in range(NCORES):
        n, g = divmod(c, GROUPS)
        cols = slice(g * DG, (g + 1) * DG)
        wire = np.float32
        if MM_DTYPE == BF16:
            import ml_dtypes
            wire = ml_dtypes.bfloat16
        in_maps.append({
            "xT": np.ascontiguousarray(np.asarray(x[n], np.float32).T).astype(wire),
            "wq": np.ascontiguousarray(np.asarray(Wq, np.float32)[:, cols]).astype(wire),
            "wk": np.ascontiguousarray(np.asarray(Wk, np.float32)[:, cols]).astype(wire),
            "wv": np.ascontiguousarray(np.asarray(Wv, np.float32)[:, cols]).astype(wire),
            "wo": np.ascontiguousarray(np.asarray(Wo, np.float32)[cols, :]).astype(wire),
        })
    return in_maps


def kernel(x, Wv, Wk, Wq, Wo, bo):
    global LAST_RESULTS
    x = np.asarray(x, np.float32)
    in_maps = shard_inputs(x, Wv, Wk, Wq, Wo)

    if "nc" not in _CACHED_NC:
        _CACHED_NC["nc"] = build_nc()
    nc = _CACHED_NC["nc"]

    trace = os.environ.get("MHA_TRACE", "0") == "1"
    res = bass_utils.run_bass_kernel_spmd(
        nc, in_maps, core_ids=list(range(NCORES)), trace=trace)
    LAST_RESULTS = res

    bo = np.asarray(bo, np.float32)
    out = np.empty((NB, SEQ, EMBED), np.float32)
    for n in range(NB):
        acc = res.results[n * GROUPS]["y"].astype(np.float32).copy()
        for g in range(1, GROUPS):
            acc += res.results[n * GROUPS + g]["y"]
        out[n] = acc + bo[None, :]
    return out


# revision 28
# speedup vs baseline: 3.2725x; 1.0000x over previous
"""Multi-head self-attention Trainium2 kernel (8-core SPMD, full IO).

Problem: x:(2,2048,1024) f32; Wq/Wk/Wv/Wo:(1024,1024); bo:(1024,)
  out = softmax((xWq)(xWk)^T / 8) (xWv) reshaped @ Wo + bo

Sharding: data parallel on batch N=2 x tensor parallel on 16 heads in
4 groups of 4 heads.  Core c handles batch c//4, heads [4*(c%4), 4*(c%4)+4).
Each core computes a partial fc_out product (2048,1024); the host sums the
4 head-group partials per batch and adds the bias.

On-chip layout (per core):
  xT   (1024,2048)  x[n]^T, embed on partitions (8 chunks of 128)
  Q^T/K^T stored as [128, 2, 2048] (dims-chunk on partitions, tokens free)
  V    stored as [128(tokens), 16, 4, 65]; col 64 = ones (denominator trick)
  scores are computed TRANSPOSED: S^T[k,q] so that exp runs on ACT and the
  softmax denominator falls out of the ones-column of V during the O^T
  accumulation (row 64 of the [65,512] psum).  No max subtraction: scores
  are ~N(0,1), bounded well inside fp32 exp range (as in the reference,
  which subtracts max only for stability, not value).
"""

import os

import numpy as np

import concourse.bass as bass
import concourse.tile as tile
from concourse import bacc, mybir
from concourse import bass_utils

F32 = mybir.dt.float32

EMBED = 1024
SEQ = 2048
NB = 2  # batch
HEADS = 16
D = 64  # head dim
NCORES = 8
GROUPS = 4  # head groups (tensor parallel)
HG = HEADS // GROUPS  # heads per core = 4
DG = HG * D  # dims per core = 256

# matmul operand dtype:
#   float32  - exact, 1/4 PE rate
#   float32r - tf32-class (~3e-4 rel), ~2 cyc/row (4-byte stream bound)
#   bfloat16 - ~5e-3 rel, full PE rate, half DMA/SBUF footprint
_MM_DTYPE_NAME = os.environ.get("MHA_MM_DTYPE", "bfloat16")
MM_DTYPE = getattr(mybir.dt, _MM_DTYPE_NAME)
BF16 = mybir.dt.bfloat16

# set by run_cores(); test.py reads exec_time_ns from here
LAST_RESULTS = None
_CACHED_NC = {}


MD = MM_DTYPE  # dtype of matmul-feeding tiles
# DRAM dtype of the big inputs: bf16 inputs are converted host-side (DMA
# cannot cast); f32r shares fp32 bits so DRAM stays f32 + bitcast at DMA.
IN_DT = BF16 if MM_DTYPE == BF16 else F32
IN_NP = None  # numpy dtype for host conversion, set below


def _in_cast(ap):
    """DRAM-side view of an input AP in the matmul dtype."""
    return ap if MD in (F32, BF16) else ap.bitcast(MD)


def build_nc():
    nc = bacc.Bacc("TRN2", target_bir_lowering=False, debug=False,
                   num_devices=NCORES)

    xT = nc.dram_tensor("xT", (EMBED, SEQ), IN_DT, kind="ExternalInput").ap()
    wq = nc.dram_tensor("wq", (EMBED, DG), IN_DT, kind="ExternalInput").ap()
    wk = nc.dram_tensor("wk", (EMBED, DG), IN_DT, kind="ExternalInput").ap()
    wv = nc.dram_tensor("wv", (EMBED, DG), IN_DT, kind="ExternalInput").ap()
    wo = nc.dram_tensor("wo", (DG, EMBED), IN_DT, kind="ExternalInput").ap()
    y = nc.dram_tensor("y", (SEQ, EMBED), F32, kind="ExternalOutput").ap()
    # DRAM bounce buffers for the softmax denominators: SBUF sources can't be
    # partition-broadcast by DMA, DRAM sources can.
    den_dram = nc.dram_tensor("den_scratch", (HG, SEQ), F32).ap()
    rden_dram = nc.dram_tensor("rden_scratch", (HG, SEQ), F32).ap()

    KC = EMBED // 128  # 8 contraction chunks for projections

    with tile.TileContext(nc) as tc:
        with (
            tc.tile_pool(name="weights", bufs=1) as wpool,
            tc.tile_pool(name="qk", bufs=1) as qkpool,
            tc.tile_pool(name="vpool", bufs=1) as vpool,
            tc.tile_pool(name="otpool", bufs=1) as otpool,
            tc.tile_pool(name="xchunk", bufs=2) as xpool,
            tc.tile_pool(name="epool", bufs=12) as epool,
            tc.tile_pool(name="stage", bufs=4) as stpool,
            tc.tile_pool(name="den", bufs=1) as denpool,
            tc.tile_pool(name="rbc", bufs=2) as rbcpool,
            tc.tile_pool(name="ystage", bufs=3) as ypool,
            tc.tile_pool(name="psum", bufs=2, space="PSUM") as pspool,
            tc.tile_pool(name="psum_o", bufs=2, space="PSUM") as popool,
        ):
            # ---- load weights ----
            wq_sb = wpool.tile([128, KC, DG], MD)
            wk_sb = wpool.tile([128, KC, DG], MD)
            wv_sb = wpool.tile([128, KC, DG], MD)
            wo_sb = wpool.tile([128, DG // 128, EMBED], MD)
            nc.sync.dma_start(out=wq_sb, in_=_in_cast(wq).rearrange("(c p) n -> p c n", p=128))
            nc.sync.dma_start(out=wk_sb, in_=_in_cast(wk).rearrange("(c p) n -> p c n", p=128))
            nc.sync.dma_start(out=wv_sb, in_=_in_cast(wv).rearrange("(c p) n -> p c n", p=128))
            nc.sync.dma_start(out=wo_sb, in_=_in_cast(wo).rearrange("(c p) n -> p c n", p=128))

            # per-512-token-chunk tiles: finer dependency granularity lets
            # phase-2 attention start as soon as its chunks are projected
            QTs = [qkpool.tile([128, 2, 512], MD, name=f"qt{t}", tag=f"qt{t}")
                   for t in range(4)]
            KTs = [qkpool.tile([128, 2, 512], MD, name=f"kt{t}", tag=f"kt{t}")
                   for t in range(4)]
            Vs = [vpool.tile([128, 4, HG, D + 1], MD, name=f"v{t}", tag=f"v{t}")
                  for t in range(4)]
            for t in range(4):
                ones_col = Vs[t][:, :, :, D:D + 1]
                nc.vector.memset(
                    ones_col.bitcast(F32) if MD == mybir.dt.float32r
                    else ones_col, 1.0)

            xTr = _in_cast(xT).rearrange("(c p) s -> p c s", p=128)

            # ---- phase 1: projections, one 256-token chunk at a time ----
            TCH = 512
            for tcb in range(SEQ // TCH):
                xc = xpool.tile([128, KC, TCH], MD)
                nc.sync.dma_start(out=xc, in_=xTr[:, :, tcb * TCH:(tcb + 1) * TCH])

                # Q^T and K^T chunks: [dims 128, tokens TCH]
                for wsb, dst in ((wq_sb, QTs), (wk_sb, KTs)):
                    for mt in range(2):
                        ps = popool.tile([128, 512], F32, name="ps1", tag="po")
                        for kc in range(KC):
                            nc.tensor.matmul(
                                ps[:, 0:TCH],
                                wsb[:, kc, mt * 128:(mt + 1) * 128],
                                xc[:, kc, :],
                                start=(kc == 0),
                                stop=(kc == KC - 1),
                            )
                        nc.vector.tensor_copy(
                            out=dst[tcb][:, mt, :], in_=ps[:, 0:TCH])

                # V chunks: [tokens 128, dims 256]
                for ti in range(TCH // 128):
                    tt = tcb * (TCH // 128) + ti
                    ps = popool.tile([128, 512], F32, name="ps1", tag="po")
                    for kc in range(KC):
                        nc.tensor.matmul(
                            ps[:, 0:DG],
                            xc[:, kc, ti * 128:(ti + 1) * 128],
                            wv_sb[:, kc, :],
                            start=(kc == 0),
                            stop=(kc == KC - 1),
                        )
                    nc.vector.tensor_copy(
                        out=Vs[tcb][:, ti, :, 0:D],
                        in_=ps[:, 0:DG].rearrange("p (h d) -> p h d", h=HG))

            # ---- phase 2: attention (scores transposed, head pairs) ----
            # OT2[p, hm, q]: partition p = 64*j + d for head h = 2*hm + j.
            # This matches wo_sb's row layout so fc_out contracts K=128/pair.
            OT2 = otpool.tile([128, 2, SEQ], MD)
            QC = 1024  # q-chunk: one [128, QC] psum = 2 banks, one exp inst

            for hm in range(2):
                for qc in range(SEQ // QC):
                    qs = slice(qc * QC, (qc + 1) * QC)
                    po = [popool.tile([D + 1, QC], F32, name="po", tag="po")
                          for _ in range(2)]
                    for m in range(SEQ // 128):
                        es = []
                        for j in range(2):  # paired heads -> concurrent MMs
                            ps = pspool.tile([128, QC], F32)
                            for ha in range(QC // 512):
                                nc.tensor.matmul(
                                    ps[:, ha * 512:(ha + 1) * 512],
                                    KTs[m // 4][j * D:(j + 1) * D, hm,
                                                (m % 4) * 128:
                                                (m % 4 + 1) * 128],
                                    QTs[2 * qc + ha][j * D:(j + 1) * D, hm, :],
                                    start=True,
                                    stop=True,
                                )
                            e = epool.tile([128, QC], MD)
                            nc.scalar.activation(
                                out=e, in_=ps,
                                func=mybir.ActivationFunctionType.Exp,
                                scale=1.0 / np.sqrt(D),
                            )
                            es.append(e)
                        for j in range(2):
                            for ha in range(QC // 512):
                                nc.tensor.matmul(
                                    po[j][:, ha * 512:(ha + 1) * 512],
                                    Vs[m // 4][:, m % 4, 2 * hm + j, :],
                                    es[j][:, ha * 512:(ha + 1) * 512],
                                    start=(m == 0),
                                    stop=(m == SEQ // 128 - 1),
                                )
                    for j in range(2):
                        h = 2 * hm + j
                        st = stpool.tile([D + 1, QC], F32)
                        nc.vector.tensor_copy(out=st, in_=po[j])
                        ot_dst = OT2[j * D:(j + 1) * D, hm, qs]
                        if MD == BF16:
                            nc.gpsimd.dma_start(out=ot_dst, in_=st[0:D, :])
                        elif MD == F32:
                            nc.sync.dma_start(out=ot_dst, in_=st[0:D, :])
                        else:
                            nc.sync.dma_start(
                                out=ot_dst, in_=st[0:D, :].bitcast(MD))
                        nc.sync.dma_start(
                            out=den_dram[h:h + 1, qs], in_=st[D:D + 1, :])

            # reciprocal, reshaped to use all 128 partitions (free dim 64)
            rsm = denpool.tile([128, HG * SEQ // 128], F32)
            den_r = den_dram.rearrange("h (a b) -> (h a) b", a=32)
            rden_r = rden_dram.rearrange("h (a b) -> (h a) b", a=32)
            nc.sync.dma_start(out=rsm, in_=den_r)
            nc.vector.reciprocal(out=rsm, in_=rsm)
            nc.sync.dma_start(out=rden_r, in_=rsm)

            # normalize O^T rows by 1/denominator (broadcast across partitions)
            for hm in range(2):
                rb = rbcpool.tile([128, SEQ], F32)
                for j in range(2):
                    nc.sync.dma_start(
                        out=rb[j * D:(j + 1) * D, :],
                        in_=rden_dram[2 * hm + j:2 * hm + j + 1, :]
                        .to_broadcast((D, SEQ)))
                nc.vector.tensor_mul(OT2[:, hm, :], OT2[:, hm, :], rb)

            # ---- phase 3: partial fc_out  y = sum_h O_h @ Wo_h (K=128/pair) ----
            for tt in range(SEQ // 128):
                for nch in range(EMBED // 512):
                    ps = pspool.tile([128, QC], F32)
                    for hm in range(2):
                        nc.tensor.matmul(
                            ps[:, 0:512],
                            OT2[:, hm, tt * 128:(tt + 1) * 128],
                            wo_sb[:, hm, nch * 512:(nch + 1) * 512],
                            start=(hm == 0),
                            stop=(hm == 1),
                        )
                    ys = ypool.tile([128, 512], F32)
                    nc.vector.tensor_copy(out=ys, in_=ps[:, 0:512])
                    nc.sync.dma_start(
                        out=y[tt * 128:(tt + 1) * 128, nch * 512:(nch + 1) * 512],
                        in_=ys)

    nc.compile()
    return nc


def shard_inputs(x, Wv, Wk, Wq, Wo):
    """Build the 8 per-core input maps."""
    in_maps = []
    for c in range(NCORES):
        n, g = divmod(c, GROUPS)
        cols = slice(g * DG, (g + 1) * DG)
        wire = np.float32
        if MM_DTYPE == BF16:
            import ml_dtypes
            wire = ml_dtypes.bfloat16
        in_maps.append({
            "xT": np.ascontiguousarray(np.asarray(x[n], np.float32).T).astype(wire),
            "wq": np.ascontiguousarray(np.asarray(Wq, np.float32)[:, cols]).astype(wire),
            "wk": np.ascontiguousarray(np.asarray(Wk, np.float32)[:, cols]).astype(wire),
            "wv": np.ascontiguousarray(np.asarray(Wv, np.float32)[:, cols]).astype(wire),
            "wo": np.ascontiguousarray(np.asarray(Wo, np.float32)[cols, :]).astype(wire),
        })
    return in_maps


def kernel(x, Wv, Wk, Wq, Wo, bo):
    global LAST_RESULTS
    x = np.asarray(x, np.float32)
    in_maps = shard_inputs(x, Wv, Wk, Wq, Wo)

    if "nc" not in _CACHED_NC:
        _CACHED_NC["nc"] = build_nc()
    nc = _CACHED_NC["nc"]

    trace = os.environ.get("MHA_TRACE", "0") == "1"
    res = bass_utils.run_bass_kernel_spmd(
        nc, in_maps, core_ids=list(range(NCORES)), trace=trace)
    LAST_RESULTS = res

    bo = np.asarray(bo, np.float32)
    out = np.empty((NB, SEQ, EMBED), np.float32)
    for n in range(NB):
        acc = res.results[n * GROUPS]["y"].astype(np.float32).copy()
        for g in range(1, GROUPS):
            acc += res.results[n * GROUPS + g]["y"]
        out[n] = acc + bo[None, :]
    return out
